# revision 4
# baseline (speedup 1.0000x reference)
"""Trainium2 Bass kernel for a 3-layer MHA encoder (EntityEncoder).

Strategy: pure data parallelism — batch B=8, one batch element per NeuronCore,
weights replicated, no collectives.

Per-core layout strategy (S=1024 seq, E=256 in-dim, D=1024 model dim, 8 heads
x 128 head-dim):
  - Activations are kept transposed X^T [D, S] in SBUF so every matmul
    contracts over the partition dim naturally.
  - Q^T, K^T computed per head ([128, S]); attention scores are computed
    directly in transposed [k, q] orientation (lhsT = K^T_h slice), so
    softmax needs no transposes anywhere.
  - exp() runs on the Scalar engine PSUM->SBUF with the 1/sqrt(128) scale
    folded in; softmax denominators come from a ones-matmul on the Tensor
    engine (which also yields the partition-broadcast of the row sums for
    free); normalization is applied to P~@V output via reciprocal+multiply
    on the Vector engine.
  - V is computed in natural [S, D] layout (lhsT = X^T s-tiles).
  - Output projection of the last layer is computed in natural [S, D]
    orientation so the final DMA to DRAM is contiguous.
  - All matmul operands use float32r (full PE rate at moving dim >= 256,
    ~2e-4 matmul precision).
"""

import sys

for _p in ("/opt/trn_rl_repo", "/root/.axon_site/_ro/trn_rl_repo"):
    if _p not in sys.path:
        sys.path.append(_p)

from contextlib import ExitStack

import numpy as np

from concourse import bacc
import concourse.mybir as mybir
from concourse.tile import TileContext
from concourse.bass_utils import run_bass_kernel_spmd

f32 = mybir.dt.float32
f32r = mybir.dt.float32r

S = 1024
E = 256
D = 1024
H = 8
HD = 128
N_CORES = 8
INV_SQRT_HD = 1.0 / float(np.sqrt(np.float64(np.float32(HD))))
AF = mybir.ActivationFunctionType
ALU = mybir.AluOpType


def _emit_kernel(nc, tc, ctx, prm, out):
    sync = nc.sync

    xt_pool = ctx.enter_context(tc.tile_pool(name="xt", bufs=2))
    v_pool = ctx.enter_context(tc.tile_pool(name="v", bufs=1))
    attw_pool = ctx.enter_context(tc.tile_pool(name="attw", bufs=1))
    qt_pool = ctx.enter_context(tc.tile_pool(name="qt", bufs=1))
    kt_pool = ctx.enter_context(tc.tile_pool(name="kt", bufs=1))
    pt_pool = ctx.enter_context(tc.tile_pool(name="pt", bufs=2))
    wvq_pool = ctx.enter_context(tc.tile_pool(name="wvq", bufs=2))
    wstrip_pool = ctx.enter_context(tc.tile_pool(name="wstrip", bufs=3))
    rep_pool = ctx.enter_context(tc.tile_pool(name="rep", bufs=1))
    bias_pool = ctx.enter_context(tc.tile_pool(name="bias", bufs=2))
    ones_pool = ctx.enter_context(tc.tile_pool(name="ones", bufs=1))
    invr_pool = ctx.enter_context(tc.tile_pool(name="invr", bufs=1))
    outsb_pool = ctx.enter_context(tc.tile_pool(name="outsb", bufs=3))

    psA = ctx.enter_context(tc.tile_pool(name="psA", bufs=2, space="PSUM"))
    psS = ctx.enter_context(tc.tile_pool(name="psS", bufs=3, space="PSUM"))
    psT = ctx.enter_context(tc.tile_pool(name="psT", bufs=2, space="PSUM"))
    psR = ctx.enter_context(tc.tile_pool(name="psR", bufs=1, space="PSUM"))

    ones_t = ones_pool.tile([128, 128], f32r, tag="ones")
    sync.dma_start(out=ones_t[:], in_=prm["ones"][:])

    # layer 0 input: x^T [E, S] -> [128, 2, 1024]
    xt0 = xt_pool.tile([128, E // 128, S], f32r, tag="xt")
    sync.dma_start(out=xt0[:], in_=prm["xt"].ap().rearrange("(k p) s -> p k s", p=128))

    xt_t = xt0
    for layer in range(3):
        kin = (E if layer == 0 else D) // 128
        last = layer == 2
        wq = prm[f"wq{layer}"].ap().rearrange("(k p) j -> p k j", p=128)
        wk = prm[f"wk{layer}"].ap().rearrange("(k p) j -> p k j", p=128)
        wv = prm[f"wv{layer}"].ap().rearrange("(k p) j -> p k j", p=128)
        wo = prm[f"wo{layer}"].ap().rearrange("(k p) j -> p k j", p=128)

        bq_t = bias_pool.tile([128, H], f32, tag="bq")
        bk_t = bias_pool.tile([128, H], f32, tag="bk")
        sync.dma_start(out=bq_t[:], in_=prm[f"bq{layer}"][:])
        sync.dma_start(out=bk_t[:], in_=prm[f"bk{layer}"][:])
        bv_t = rep_pool.tile([128, D], f32, tag="rep")
        sync.dma_start(out=bv_t[:], in_=prm[f"bv{layer}"][:])
        if not last:
            bo_t = bias_pool.tile([128, H], f32, tag="bo")
            sync.dma_start(out=bo_t[:], in_=prm[f"bo{layer}"][:])
        else:
            bo2_t = rep_pool.tile([128, D], f32, tag="rep")
            sync.dma_start(out=bo2_t[:], in_=prm["bo2r"][:])

        # ---- Phase 1: V = relu(X @ Wv + bv), natural [S, D] layout ----
        v_sb = v_pool.tile([128, 8, D], f32r, tag="v")
        for qtr in range(4):
            jr = slice(qtr * 256, (qtr + 1) * 256)
            wvq = wvq_pool.tile([128, kin, 256], f32r, tag="wvq")
            sync.dma_start(out=wvq[:], in_=wv[:, :, jr])
            for st in range(8):
                psf = psA.tile([128, 512], f32, tag="psA", name=f"psA_v{layer}_{qtr}_{st}")
                ps = psf[:, :256]
                for kt in range(kin):
                    nc.tensor.matmul(
                        ps,
                        xt_t[:, kt, st * 128 : (st + 1) * 128],
                        wvq[:, kt, :],
                        start=(kt == 0),
                        stop=(kt == kin - 1),
                    )
                vsl = v_sb[:, st, jr]
                nc.vector.tensor_tensor(vsl, ps, bv_t[:, jr], ALU.add)
                nc.vector.tensor_scalar(vsl, vsl, 0.0, None, op0=ALU.max)

        # ---- Phase 2: per-head attention, fully transposed ----
        attw = attw_pool.tile([128, H, S], f32r, tag="attw")

        def emit_proj(h):
            """Q^T_h, K^T_h = relu(W[:, h]^T @ X^T + b) -> [128, S]."""
            wqs = wstrip_pool.tile([128, kin, 128], f32r, tag="wstrip")
            sync.dma_start(out=wqs[:], in_=wq[:, :, h * 128 : (h + 1) * 128])
            wks = wstrip_pool.tile([128, kin, 128], f32r, tag="wstrip")
            sync.dma_start(out=wks[:], in_=wk[:, :, h * 128 : (h + 1) * 128])
            qt_t = qt_pool.tile([128, S], f32r, tag="qt")
            kt_t = kt_pool.tile([128, S], f32r, tag="kt")
            for wsb, dst, b_t in ((wqs, qt_t, bq_t), (wks, kt_t, bk_t)):
                for sr in range(2):
                    ps = psA.tile([128, 512], f32, tag="psA")
                    for kt in range(kin):
                        nc.tensor.matmul(
                            ps,
                            wsb[:, kt, :],
                            xt_t[:, kt, sr * 512 : (sr + 1) * 512],
                            start=(kt == 0),
                            stop=(kt == kin - 1),
                        )
                    nc.scalar.activation(
                        dst[:, sr * 512 : (sr + 1) * 512],
                        ps,
                        AF.Relu,
                        bias=b_t[:, h : h + 1],
                        scale=1.0,
                    )
            return qt_t, kt_t

        def emit_scores_exp(qt_t, kt_t, qr):
            """P~^T[k, q-range] = exp(K^T_h.T-slices @ Q^T_h * inv_sqrt_hd)."""
            pt_t = pt_pool.tile([128, 8, 512], f32r, tag="pt")
            for kt in range(8):
                ps = psS.tile([128, 512], f32, tag="psS")
                nc.tensor.matmul(
                    ps,
                    kt_t[:, kt * 128 : (kt + 1) * 128],
                    qt_t[:, qr * 512 : (qr + 1) * 512],
                    start=True,
                    stop=True,
                )
                nc.scalar.activation(
                    pt_t[:, kt, :], ps, AF.Exp, bias=0.0, scale=INV_SQRT_HD
                )
            return pt_t

        def emit_r_att_norm(h, qr, pt_t):
            """row-sums (ones matmul), att^T_h = V_h^T-slices @ P~^T, normalize."""
            psr = psR.tile([128, 512], f32, tag="psR")
            for kt in range(8):
                nc.tensor.matmul(
                    psr, ones_t[:], pt_t[:, kt, :], start=(kt == 0), stop=(kt == 7)
                )
            pst = psT.tile([128, 512], f32, tag="psT")
            for kt in range(8):
                nc.tensor.matmul(
                    pst,
                    v_sb[:, kt, h * 128 : (h + 1) * 128],
                    pt_t[:, kt, :],
                    start=(kt == 0),
                    stop=(kt == 7),
                )
            invr = invr_pool.tile([128, 512], f32, tag="invr")
            nc.vector.reciprocal_approx_fast(out=invr[:], in_=psr[:])
            nc.vector.tensor_tensor(
                attw[:, h, qr * 512 : (qr + 1) * 512], pst, invr[:], ALU.mult
            )

        prev = None
        for h in range(H):
            qt_t, kt_t = emit_proj(h)
            for qr in range(2):
                pt_t = emit_scores_exp(qt_t, kt_t, qr)
                if prev is not None:
                    emit_r_att_norm(*prev)
                prev = (h, qr, pt_t)
        emit_r_att_norm(*prev)

        # ---- Phase 3: output projection ----
        if not last:
            # O^T[j, s] = relu(Wo[:, j]^T @ att^T + bo[j]); becomes next X^T
            ot = xt_pool.tile([128, 8, S], f32r, tag="xt")
            for j in range(8):
                wos = wstrip_pool.tile([128, 8, 128], f32r, tag="wstrip")
                sync.dma_start(out=wos[:], in_=wo[:, :, j * 128 : (j + 1) * 128])
                for sr in range(2):
                    ps = psA.tile([128, 512], f32, tag="psA")
                    for it in range(8):
                        nc.tensor.matmul(
                            ps,
                            wos[:, it, :],
                            attw[:, it, sr * 512 : (sr + 1) * 512],
                            start=(it == 0),
                            stop=(it == 7),
                        )
                    nc.scalar.activation(
                        ot[:, j, sr * 512 : (sr + 1) * 512],
                        ps,
                        AF.Relu,
                        bias=bo_t[:, j : j + 1],
                        scale=1.0,
                    )
            xt_t = ot
        else:
            # O[s, j] = relu(att @ Wo + bo), natural layout -> contiguous DMA
            for qtr in range(4):
                jr = slice(qtr * 256, (qtr + 1) * 256)
                wo2q = wvq_pool.tile([128, 8, 256], f32r, tag="wvq")
                sync.dma_start(out=wo2q[:], in_=wo[:, :, jr])
                for st in range(8):
                    psf = psA.tile([128, 512], f32, tag="psA", name=f"psA_o2_{qtr}_{st}")
                    ps = psf[:, :256]
                    for it in range(8):
                        nc.tensor.matmul(
                            ps,
                            attw[:, it, st * 128 : (st + 1) * 128],
                            wo2q[:, it, :],
                            start=(it == 0),
                            stop=(it == 7),
                        )
                    osb = outsb_pool.tile([128, 256], f32, tag="outsb")
                    nc.vector.tensor_tensor(osb[:], ps, bo2_t[:, jr], ALU.add)
                    nc.vector.tensor_scalar(osb[:], osb[:], 0.0, None, op0=ALU.max)
                    sync.dma_start(
                        out=out[st * 128 : (st + 1) * 128, jr], in_=osb[:]
                    )


def _build():
    nc = bacc.Bacc()
    prm = {}

    def inp(name, shape, dt=f32r):
        prm[name] = nc.declare_dram_parameter(name, list(shape), dt, isOutput=False)

    inp("xt", [E, S])
    for layer in range(3):
        din = E if layer == 0 else D
        inp(f"wq{layer}", [din, D])
        inp(f"wk{layer}", [din, D])
        inp(f"wv{layer}", [din, D])
        inp(f"wo{layer}", [D, D])
        inp(f"bq{layer}", [128, H], f32)
        inp(f"bk{layer}", [128, H], f32)
        inp(f"bv{layer}", [128, D], f32)
        if layer < 2:
            inp(f"bo{layer}", [128, H], f32)
    inp("bo2r", [128, D], f32)
    inp("ones", [128, 128])
    out = nc.declare_dram_parameter("out", [S, D], f32, isOutput=True)

    with TileContext(nc) as tc, ExitStack() as ctx:
        _emit_kernel(nc, tc, ctx, prm, out)
    nc.compile()
    return nc


_NC = None


def _get_nc():
    global _NC
    if _NC is None:
        _NC = _build()
    return _NC


def _pt8(b):
    # [1024] -> per-partition layout [128, 8]: out[p, t] = b[t*128 + p]
    return np.ascontiguousarray(b.reshape(H, 128).T.astype(np.float32))


def _rep(b):
    return np.ascontiguousarray(np.broadcast_to(b.astype(np.float32), (128, D)))


def make_in_maps(x, Wq0, bq0, Wk0, bk0, Wv0, bv0, Wq, bq, Wk, bk, Wv, bv, Wo, bo):
    shared = {"ones": np.ones((128, 128), np.float32)}
    for layer in range(3):
        if layer == 0:
            wql, bql, wkl, bkl, wvl, bvl = Wq0, bq0, Wk0, bk0, Wv0, bv0
        else:
            i = layer - 1
            wql, bql, wkl, bkl, wvl, bvl = Wq[i], bq[i], Wk[i], bk[i], Wv[i], bv[i]
        shared[f"wq{layer}"] = np.ascontiguousarray(wql, np.float32)
        shared[f"wk{layer}"] = np.ascontiguousarray(wkl, np.float32)
        shared[f"wv{layer}"] = np.ascontiguousarray(wvl, np.float32)
        shared[f"wo{layer}"] = np.ascontiguousarray(Wo[layer], np.float32)
        shared[f"bq{layer}"] = _pt8(bql)
        shared[f"bk{layer}"] = _pt8(bkl)
        shared[f"bv{layer}"] = _rep(bvl)
        if layer < 2:
            shared[f"bo{layer}"] = _pt8(bo[layer])
    shared["bo2r"] = _rep(bo[2])

    in_maps = []
    for b in range(N_CORES):
        m = dict(shared)
        m["xt"] = np.ascontiguousarray(np.asarray(x[b]).T, np.float32)
        in_maps.append(m)
    return in_maps


def kernel(**inputs):
    nc = _get_nc()
    in_maps = make_in_maps(**inputs)
    res = run_bass_kernel_spmd(nc, in_maps, list(range(N_CORES)))
    return np.stack([res.results[b]["out"] for b in range(N_CORES)]).astype(np.float32)
